# revision 3
# baseline (speedup 1.0000x reference)
"""Trainium2 Bass kernel for the attention-LSTM captioner (teacher forcing).

Sharding: data-parallel over batch across 8 cores (16 rows each), one SPMD
kernel launch, no collectives. Per core the logits matmul is batched over all
19 timesteps so logit_k streams from HBM once.
"""
import numpy as np
import ml_dtypes
from contextlib import ExitStack

import concourse.bass as bass
import concourse.tile as tile
import concourse.mybir as mybir
from concourse import bacc
from concourse.bass import ts, ds
from concourse.bass_utils import run_bass_kernel_spmd
from concourse.masks import make_identity

B, L, F, U, H, ED, VOC, T = 128, 64, 2048, 512, 512, 256, 10000, 20
NCORE = 8
BC = B // NCORE            # 16 batch rows per core
BL = BC * L                # 1024
T1 = T - 1                 # 19 steps
NX = 304                   # T1 * BC rows of X per core
XD = ED + F + H            # 2816
KX = XD // 128             # 22 k-tiles for logits
START_TOK = 1

BF = mybir.dt.bfloat16
F32 = mybir.dt.float32
AF = mybir.ActivationFunctionType
ALU = mybir.AluOpType

_cache = {}


def _build_program():
    nc = bacc.Bacc(None, target_bir_lowering=False, debug=False, num_devices=NCORE)

    d = {}
    def din(name, shape, dt):
        d[name] = nc.dram_tensor(name, shape, dt, kind="ExternalInput").ap()
    din("imgT", [F, BL], BF)          # img_flat.T  (per core)
    din("imgN", [BL, F], BF)          # img_flat    (per core)
    din("widx", [NX, 1], mybir.dt.int32)  # word index per X-row (t*16+b)
    din("emb", [VOC, ED], BF)
    din("lstmk", [ED + F, 4 * H], BF)
    din("lstmrk", [H, 4 * H], BF)
    din("lstmbB", [BC, 4 * H], F32)
    din("w1k", [F, U], BF)
    din("w1bT", [128, U // 128], F32)
    din("w2k", [H, U], BF)
    din("w2bT", [128, U // 128], F32)
    din("vkT", [128, U // 128], BF)
    din("fbkT", [128, H // 128], BF)
    din("fbb", [BC, 1], F32)
    din("logitk", [XD, VOC], BF)
    din("logitbB", [128, VOC], BF)
    din("ih0k", [F, H], BF)
    din("ih0bT", [128, H // 128], F32)
    din("ih1k", [H, H], BF)
    din("ih1bT", [128, H // 128], F32)
    din("ic0k", [F, H], BF)
    din("ic0bT", [128, H // 128], F32)
    din("ic1k", [H, H], BF)
    din("ic1bT", [128, H // 128], F32)
    out_d = nc.dram_tensor("out", [NX, VOC], F32, kind="ExternalOutput").ap()

    KF = F // 128   # 16
    KH = H // 128   # 4
    KE = ED // 128  # 2
    KBL = BL // 128 # 8

    with tile.TileContext(nc) as tc, ExitStack() as ctx:
        # ---- long-lived pools ----
        const = ctx.enter_context(tc.tile_pool(name="const", bufs=1))
        state = ctx.enter_context(tc.tile_pool(name="state", bufs=2))
        p2w = ctx.enter_context(tc.tile_pool(name="p2w", bufs=2))

        ident = const.tile([128, 128], BF, tag="ident")
        make_identity(nc, ident)

        # resident weights
        w2k_t = const.tile([128, KH, U], BF, tag="w2k")
        nc.sync.dma_start(w2k_t[:], d["w2k"].rearrange("(k p) m -> p k m", p=128))
        rk_t = const.tile([128, KH, 4 * H], BF, tag="rk")
        nc.sync.dma_start(rk_t[:], d["lstmrk"].rearrange("(k p) m -> p k m", p=128))
        we_t = const.tile([128, KE, 4 * H], BF, tag="we")
        nc.sync.dma_start(we_t[:], d["lstmk"][0:ED, :].rearrange("(k p) m -> p k m", p=128))
        vk_t = const.tile([128, KH, 1], BF, tag="vk")
        nc.sync.dma_start(vk_t[:, :, 0], d["vkT"])
        fbk_t = const.tile([128, KH, 1], BF, tag="fbk")
        nc.sync.dma_start(fbk_t[:, :, 0], d["fbkT"])
        w1b_t = const.tile([128, KH], F32, tag="w1b")
        nc.sync.dma_start(w1b_t[:], d["w1bT"])
        w2b_t = const.tile([128, KH], F32, tag="w2b")
        nc.sync.dma_start(w2b_t[:], d["w2bT"])
        lstmb_t = const.tile([BC, 4 * H], F32, tag="lstmb")
        nc.sync.dma_start(lstmb_t[:], d["lstmbB"])
        fbb_t = const.tile([BC, 1], F32, tag="fbb")
        nc.sync.dma_start(fbb_t[:], d["fbb"])
        ih1k_t = const.tile([128, KH, H], BF, tag="ih1k")
        nc.sync.dma_start(ih1k_t[:], d["ih1k"].rearrange("(k p) m -> p k m", p=128))
        ic1k_t = const.tile([128, KH, H], BF, tag="ic1k")
        nc.sync.dma_start(ic1k_t[:], d["ic1k"].rearrange("(k p) m -> p k m", p=128))
        ih0b_t = const.tile([128, KH], F32, tag="ih0b")
        nc.sync.dma_start(ih0b_t[:], d["ih0bT"])
        ih1b_t = const.tile([128, KH], F32, tag="ih1b")
        nc.sync.dma_start(ih1b_t[:], d["ih1bT"])
        ic0b_t = const.tile([128, KH], F32, tag="ic0b")
        nc.sync.dma_start(ic0b_t[:], d["ic0bT"])
        ic1b_t = const.tile([128, KH], F32, tag="ic1b")
        nc.sync.dma_start(ic1b_t[:], d["ic1bT"])

        # persistent big tensors
        G_t = const.tile([128, KBL, 4 * H], BF, tag="G")           # img @ W_c
        featT = const.tile([128, KH, BL], BF, tag="featT")         # (W1.T @ imgT) + b
        Wstk = const.tile([128, KBL, NX], BF, tag="Wstk")          # block-diag attn weights
        XT = const.tile([128, KX, NX], BF, tag="XT")               # [e; ctx; h2].T
        ET = const.tile([128, KE, NX], BF, tag="ET")
        meanT = const.tile([128, KF, BC], BF, tag="meanT")

        nc.any.memset(Wstk[:], 0.0)

        # =========== precompute: featT + mean ===========
        with tc.tile_pool(name="pc_sb", bufs=3) as pcs, \
             tc.tile_pool(name="pc_ps", bufs=1, space="PSUM") as pcp:
            mean_f = pcs.tile([128, KF, BC], F32, tag="meanf")
            fps = {}
            for n2 in range(2):
                for k in range(KF):
                    it = pcs.tile([128, BL], BF, tag="imgTt")
                    nc.sync.dma_start(it[:], d["imgT"][ts(k, 128), :])
                    if n2 == 0:
                        nc.vector.reduce_sum(
                            mean_f[:, k, :],
                            it[:].rearrange("p (b l) -> p b l", b=BC),
                            axis=mybir.AxisListType.X,
                        )
                    w1t = pcs.tile([128, U], BF, tag="w1t")
                    nc.sync.dma_start(w1t[:], d["w1k"][ts(k, 128), :])
                    for m in range(KH):
                        if k == 0:
                            fps[(n2, m)] = pcp.tile([128, 512], F32, tag=f"fp{m}", name=f"fp{m}")
                        nc.tensor.matmul(
                            fps[(n2, m)][:], w1t[:, ts(m, 128)], it[:, ts(n2, 512)],
                            start=(k == 0), stop=(k == KF - 1),
                        )
                for m in range(KH):
                    nc.scalar.activation(
                        featT[:, m, ts(n2, 512)], fps[(n2, m)][:],
                        AF.Identity, bias=w1b_t[:, m : m + 1],
                    )
            nc.vector.tensor_scalar_mul(meanT[:], mean_f[:], 1.0 / L)

        # =========== precompute: G = img @ W_c ===========
        with tc.tile_pool(name="g_sb", bufs=3) as gs, \
             tc.tile_pool(name="g_ps", bufs=1, space="PSUM") as gp:
            for n in range(4):
                gps = [gp.tile([128, 512], F32, tag=f"gp{m}", name=f"gp{m}") for m in range(KBL)]
                for k in range(KF):
                    it = gs.tile([128, BL], BF, tag="imgTt")
                    nc.sync.dma_start(it[:], d["imgT"][ts(k, 128), :])
                    wct = gs.tile([128, 512], BF, tag="wct")
                    nc.sync.dma_start(wct[:], d["lstmk"][ED + k * 128 : ED + (k + 1) * 128, ts(n, 512)])
                    for m in range(KBL):
                        nc.tensor.matmul(
                            gps[m][:], it[:, ts(m, 128)], wct[:],
                            start=(k == 0), stop=(k == KF - 1),
                        )
                for m in range(KBL):
                    nc.vector.tensor_copy(G_t[:, m, ts(n, 512)], gps[m][:])

        # =========== init h / c ===========
        hT = state.tile([128, KH, BC], BF, tag="hT")
        c_st = state.tile([BC, H], F32, tag="c")
        with tc.tile_pool(name="i_sb", bufs=3) as isb, \
             tc.tile_pool(name="i_ps", bufs=1, space="PSUM") as ips:
            for which in range(2):  # 0: h, 1: c
                k0, b0, k1, b1 = (
                    ("ih0k", ih0b_t, ih1k_t, ih1b_t) if which == 0
                    else ("ic0k", ic0b_t, ic1k_t, ic1b_t)
                )
                a1 = isb.tile([128, KH, BC], BF, tag="a1")
                ps1 = [ips.tile([128, BC], F32, tag=f"ip{m}", name=f"ip{m}") for m in range(KH)]
                for k in range(KF):
                    w0t = isb.tile([128, H], BF, tag="w0t")
                    nc.sync.dma_start(w0t[:], d[k0][ts(k, 128), :])
                    for m in range(KH):
                        nc.tensor.matmul(
                            ps1[m][:], w0t[:, ts(m, 128)], meanT[:, k, :],
                            start=(k == 0), stop=(k == KF - 1),
                        )
                for m in range(KH):
                    nc.scalar.activation(a1[:, m, :], ps1[m][:], AF.Relu, bias=b0[:, m : m + 1])
                ps2 = [ips.tile([128, BC], F32, tag=f"ip{m}", name=f"iq{m}") for m in range(KH)]
                for k in range(KH):
                    for m in range(KH):
                        nc.tensor.matmul(
                            ps2[m][:], k1[:, k, ts(m, 128)], a1[:, k, :],
                            start=(k == 0), stop=(k == KH - 1),
                        )
                if which == 0:
                    for m in range(KH):
                        nc.scalar.activation(hT[:, m, :], ps2[m][:], AF.Relu, bias=b1[:, m : m + 1])
                        nc.scalar.activation(hT[:, m, :], hT[:, m, :], AF.Tanh)
                else:
                    cT = isb.tile([128, KH, BC], BF, tag="cT")
                    for m in range(KH):
                        nc.scalar.activation(cT[:, m, :], ps2[m][:], AF.Relu, bias=b1[:, m : m + 1])
                        nc.scalar.activation(cT[:, m, :], cT[:, m, :], AF.Tanh)
                    for m in range(KH):
                        tp = ips.tile([BC, 128], BF, tag="ctr")
                        nc.tensor.transpose(tp[:], cT[:, m, :], ident[:])
                        nc.vector.tensor_copy(c_st[:, ts(m, 128)], tp[:])

        # =========== embedding gather -> ET ===========
        with tc.tile_pool(name="e_sb", bufs=2) as esb, \
             tc.tile_pool(name="e_ps", bufs=2, space="PSUM") as eps:
            for r in range(3):
                rows = min(128, NX - r * 128)
                it = esb.tile([128, 1], mybir.dt.int32, tag="eidx")
                nc.sync.dma_start(it[:rows], d["widx"][ds(r * 128, rows), :])
                eg = esb.tile([128, ED], BF, tag="eg")
                nc.gpsimd.indirect_dma_start(
                    out=eg[:rows], out_offset=None,
                    in_=d["emb"][:],
                    in_offset=bass.IndirectOffsetOnAxis(ap=it[:rows, :1], axis=0),
                )
                for cc in range(KE):
                    tp = eps.tile([128, 128], BF, tag="etr")
                    nc.tensor.transpose(tp[:, :rows], eg[:rows, ts(cc, 128)], ident[:rows, :rows])
                    nc.vector.tensor_copy(ET[:, cc, ds(r * 128, rows)], tp[:, :rows])

        # =========== recurrent steps ===========
        with tc.tile_pool(name="s_sb", bufs=1) as ssb, \
             tc.tile_pool(name="s_ps", bufs=1, space="PSUM") as sps:
            for t in range(T1):
                # S1: hid_projT = W2.T @ h + b  -> [128, KH, BC] f32
                hidT = ssb.tile([128, KH, BC], BF, tag="hidT", bufs=2)
                for m in range(KH):
                    ph = sps.tile([128, BC], F32, tag="ph")
                    for k in range(KH):
                        nc.tensor.matmul(
                            ph[:], w2k_t[:, k, ts(m, 128)], hT[:, k, :],
                            start=(k == 0), stop=(k == KH - 1),
                        )
                    nc.scalar.activation(hidT[:, m, :], ph[:], AF.Identity, bias=w2b_t[:, m : m + 1])

                # S2: tanh(featT + hidT bcast over l)
                tnh = ssb.tile([128, KH, BL], BF, tag="tnh", bufs=2)
                for m in range(KH):
                    nc.vector.tensor_tensor(
                        tnh[:, m, :].rearrange("p (b l) -> p b l", b=BC),
                        featT[:, m, :].rearrange("p (b l) -> p b l", b=BC),
                        hidT[:, m, :, None].to_broadcast([128, BC, L]),
                        ALU.add,
                    )
                    nc.scalar.activation(tnh[:, m, :], tnh[:, m, :], AF.Tanh)

                # S3: scores = V.T @ tanh -> [1, BL]
                sc1 = ssb.tile([1, BL], F32, tag="sc1")
                for n2 in range(2):
                    psc = sps.tile([1, 512], F32, tag="psc")
                    for k in range(KH):
                        nc.tensor.matmul(
                            psc[:], vk_t[:, k, :], tnh[:, k, ts(n2, 512)],
                            start=(k == 0), stop=(k == KH - 1),
                        )
                    nc.vector.tensor_copy(sc1[:, ts(n2, 512)], psc[:])

                # S4: beta = sigmoid(h @ fbeta + b) -> [BC, 1]
                pb = sps.tile([BC, 1], F32, tag="pb")
                for k in range(KH):
                    nc.tensor.matmul(
                        pb[:], hT[:, k, :], fbk_t[:, k, :],
                        start=(k == 0), stop=(k == KH - 1),
                    )
                beta = ssb.tile([BC, 1], F32, tag="beta", bufs=2)
                nc.scalar.activation(beta[:], pb[:], AF.Sigmoid, bias=fbb_t[:, 0:1])

                # S5: softmax over l, fold beta/sum, write block-diag Wstk col block
                scb = ssb.tile([BC, L], F32, tag="scb", bufs=2)
                nc.sync.dma_start(scb[:], sc1[:].rearrange("o (b l) -> o b l", b=BC))
                mx = ssb.tile([BC, 1], F32, tag="mx", bufs=2)
                nc.vector.reduce_max(mx[:], scb[:], axis=mybir.AxisListType.X)
                nmx = ssb.tile([BC, 1], F32, tag="nmx", bufs=2)
                nc.vector.tensor_scalar_mul(nmx[:], mx[:], -1.0)
                ex = ssb.tile([BC, L], F32, tag="ex", bufs=2)
                sm = ssb.tile([BC, 1], F32, tag="sm", bufs=2)
                nc.scalar.activation(ex[:], scb[:], AF.Exp, bias=nmx[:, 0:1], accum_out=sm[:])
                rs = ssb.tile([BC, 1], F32, tag="rs", bufs=2)
                nc.vector.reciprocal(rs[:], sm[:])
                fac = ssb.tile([BC, 1], F32, tag="fac", bufs=2)
                nc.vector.tensor_tensor(fac[:], beta[:], rs[:], ALU.mult)
                wb = ssb.tile([BC, L], BF, tag="wb", bufs=2)
                nc.vector.tensor_scalar_mul(wb[:], ex[:], fac[:, 0:1])
                ptw = sps.tile([L, BC], BF, tag="ptw")
                nc.tensor.transpose(ptw[:], wb[:], ident[:BC, :BC])
                wlb = ssb.tile([L, BC], BF, tag="wlb", bufs=2)
                nc.vector.tensor_copy(wlb[:], ptw[:])
                # scatter into Wstk columns t*16 + b (block diagonal)
                wflat_lo = Wstk[0:64, :, :].rearrange("p k n -> p (k n)")
                wflat_hi = Wstk[64:128, :, :].rearrange("p k n -> p (k n)")
                nc.sync.dma_start(wflat_lo[:, t * BC :: NX + 2], wlb[:, 0::2])
                nc.sync.dma_start(wflat_hi[:, t * BC + 1 :: NX + 2], wlb[:, 1::2])

                # S6+S7: z = Wstk_t.T @ G + h.T @ rk + e.T @ We  (+bias) -> gates
                gate_fn = [AF.Sigmoid, AF.Sigmoid, AF.Tanh, AF.Sigmoid]
                gates = []
                for n in range(4):
                    pz = sps.tile([BC, 512], F32, tag="pz", bufs=2)
                    nmm = KBL + KH + KE
                    i_mm = 0
                    for k in range(KBL):
                        nc.tensor.matmul(
                            pz[:], Wstk[:, k, ds(t * BC, BC)], G_t[:, k, ts(n, 512)],
                            start=(i_mm == 0), stop=(i_mm == nmm - 1))
                        i_mm += 1
                    for k in range(KH):
                        nc.tensor.matmul(
                            pz[:], hT[:, k, :], rk_t[:, k, ts(n, 512)],
                            start=(i_mm == 0), stop=(i_mm == nmm - 1))
                        i_mm += 1
                    for k in range(KE):
                        nc.tensor.matmul(
                            pz[:], ET[:, k, ds(t * BC, BC)], we_t[:, k, ts(n, 512)],
                            start=(i_mm == 0), stop=(i_mm == nmm - 1))
                        i_mm += 1
                    zb = ssb.tile([BC, 512], F32, tag="zb")
                    nc.vector.tensor_add(zb[:], pz[:], lstmb_t[:, ts(n, 512)])
                    g = ssb.tile([BC, 512], F32, tag=f"gate{n}")
                    nc.scalar.activation(g[:], zb[:], gate_fn[n])
                    gates.append(g)

                sig_i, sig_f, tanh_g, sig_o = gates
                t1_ = ssb.tile([BC, H], F32, tag="t1")
                nc.vector.tensor_tensor(t1_[:], sig_f[:], c_st[:], ALU.mult)
                t2_ = ssb.tile([BC, H], F32, tag="t2")
                nc.vector.tensor_tensor(t2_[:], sig_i[:], tanh_g[:], ALU.mult)
                c_new = state.tile([BC, H], F32, tag="c")
                nc.vector.tensor_add(c_new[:], t1_[:], t2_[:])
                tc2 = ssb.tile([BC, H], F32, tag="tc2")
                nc.scalar.activation(tc2[:], c_new[:], AF.Tanh)
                h2 = ssb.tile([BC, H], BF, tag="h2", bufs=2)
                nc.vector.tensor_tensor(h2[:], sig_o[:], tc2[:], ALU.mult)

                # S8: transpose h2 -> new hT; also into XT rows 2304:2816
                hT_new = state.tile([128, KH, BC], BF, tag="hT")
                for k in range(KH):
                    pt = sps.tile([128, BC], BF, tag="pt", bufs=2)
                    nc.tensor.transpose(pt[:], h2[:, ts(k, 128)], ident[:BC, :BC])
                    nc.vector.tensor_copy(hT_new[:, k, :], pt[:])
                    nc.vector.tensor_copy(XT[:, 18 + k, ds(t * BC, BC)], pt[:])
                hT = hT_new
                c_st = c_new

        # copy ET into XT rows 0:256
        for k in range(KE):
            nc.vector.tensor_copy(XT[:, k, :], ET[:, k, :])

        # =========== CTX: XT rows 256:2304 = (img.T @ Wstk) ===========
        with tc.tile_pool(name="c_sb", bufs=2) as csb, \
             tc.tile_pool(name="c_ps", bufs=1, space="PSUM") as cps:
            for half in range(2):
                pcs_ = [cps.tile([128, NX], F32, tag=f"cp{m}", name=f"cp{m}") for m in range(8)]
                for k in range(KBL):
                    int_ = csb.tile([128, F], BF, tag="imgNt")
                    nc.sync.dma_start(int_[:], d["imgN"][ts(k, 128), :])
                    for m in range(8):
                        mg = half * 8 + m
                        nc.tensor.matmul(
                            pcs_[m][:], int_[:, ts(mg, 128)], Wstk[:, k, :],
                            start=(k == 0), stop=(k == KBL - 1),
                        )
                for m in range(8):
                    mg = half * 8 + m
                    nc.vector.tensor_copy(XT[:, 2 + mg, :], pcs_[m][:])

        # =========== logits ===========
        NV = 20
        with tc.tile_pool(name="l_ps", bufs=2, space="PSUM") as lps, \
             tc.tile_pool(name="l_sb", bufs=3) as lsb:
            logitk_r = d["logitk"].rearrange("(a p) v -> p a v", p=128)
            for v in range(NV):
                vs = min(512, VOC - v * 512)
                wv = p2w.tile([128, KX, 512], BF, tag="wv")
                nc.sync.dma_start(wv[:, :, :vs], logitk_r[:, :, ds(v * 512, vs)])
                lb = lsb.tile([128, 512], BF, tag="lb")
                nc.sync.dma_start(lb[:, :vs], d["logitbB"][:, ds(v * 512, vs)])
                for m in range(3):
                    rows = min(128, NX - m * 128)
                    pl = lps.tile([128, 512], F32, tag="pl")
                    for k in range(KX):
                        nc.tensor.matmul(
                            pl[:rows, :vs], XT[:, k, ds(m * 128, rows)], wv[:, k, :vs],
                            start=(k == 0), stop=(k == KX - 1),
                        )
                    ol = lsb.tile([128, 512], F32, tag="ol")
                    nc.vector.tensor_add(ol[:rows, :vs], pl[:rows, :vs], lb[:rows, :vs])
                    nc.sync.dma_start(
                        out_d[ds(m * 128, rows), ds(v * 512, vs)], ol[:rows, :vs]
                    )

    nc.compile()
    return nc


def _bf(x):
    return np.ascontiguousarray(np.asarray(x, dtype=np.float32)).astype(ml_dtypes.bfloat16)


def _f32(x):
    return np.ascontiguousarray(np.asarray(x, dtype=np.float32))


def kernel(
    img_tensor, target, emb, W1_k, W1_b, W2_k, W2_b, V_k, V_b,
    fbeta_k, fbeta_b, lstm_k, lstm_rk, lstm_b, logit_k, logit_b,
    ih0_k, ih0_b, ih1_k, ih1_b, ic0_k, ic0_b, ic1_k, ic1_b,
):
    if "nc" not in _cache:
        _cache["nc"] = _build_program()
    nc = _cache["nc"]

    img = _f32(img_tensor)           # (B, L, F)
    tgt = np.asarray(target)
    words = np.concatenate(
        [np.full((B, 1), START_TOK, dtype=np.int64), tgt[:, 1:-1].astype(np.int64)],
        axis=1,
    ).T  # (T1, B)

    def colT(v, kk):  # (kk*128,) -> (128, kk) fp32
        return np.ascontiguousarray(_f32(v).reshape(kk, 128).T)

    shared = {
        "emb": _bf(emb),
        "lstmk": _bf(lstm_k),
        "lstmrk": _bf(lstm_rk),
        "lstmbB": np.tile(_f32(lstm_b)[None, :], (BC, 1)),
        "w1k": _bf(W1_k),
        "w1bT": colT(W1_b, U // 128),
        "w2k": _bf(W2_k),
        "w2bT": colT(W2_b, U // 128),
        "vkT": _bf(colT(V_k.reshape(-1), U // 128)),
        "fbkT": _bf(colT(fbeta_k.reshape(-1), H // 128)),
        "fbb": np.full((BC, 1), float(np.asarray(fbeta_b).reshape(-1)[0]), np.float32),
        "logitk": _bf(logit_k),
        "logitbB": np.tile(_bf(logit_b)[None, :], (128, 1)),
        "ih0k": _bf(ih0_k), "ih0bT": colT(ih0_b, H // 128),
        "ih1k": _bf(ih1_k), "ih1bT": colT(ih1_b, H // 128),
        "ic0k": _bf(ic0_k), "ic0bT": colT(ic0_b, H // 128),
        "ic1k": _bf(ic1_k), "ic1bT": colT(ic1_b, H // 128),
    }

    in_maps = []
    for c in range(NCORE):
        imgc = img[c * BC : (c + 1) * BC].reshape(BL, F)
        wc = words[:, c * BC : (c + 1) * BC].reshape(NX, 1)  # (t, b) row-major
        m = dict(shared)
        m["imgN"] = _bf(imgc)
        m["imgT"] = _bf(imgc.T)
        m["widx"] = wc.astype(np.int32)
        in_maps.append(m)

    res = run_bass_kernel_spmd(nc, in_maps, list(range(NCORE)))

    out = np.empty((B, VOC, T1), np.float32)
    for c in range(NCORE):
        oc = res.results[c]["out"].reshape(T1, BC, VOC)
        out[c * BC : (c + 1) * BC] = oc.transpose(1, 2, 0)
    return out


# revision 17
# speedup vs baseline: 1.3905x; 1.3905x over previous
"""Trainium2 Bass kernel for the attention-LSTM captioner (teacher forcing).

Sharding: data-parallel over batch across 8 cores (16 rows each), one SPMD
kernel launch, no collectives. Inside a core the 16 rows are split into NCH
independent chains that are software-pipelined to keep the PE busy. The
logits matmul is batched over all 19 timesteps so logit_k streams once.
"""
import numpy as np
import ml_dtypes
from contextlib import ExitStack

import concourse.bass as bass
import concourse.tile as tile
import concourse.mybir as mybir
from concourse import bacc
from concourse.bass import ts, ds
from concourse.bass_utils import run_bass_kernel_spmd
from concourse.masks import make_identity

B, L, F, U, H, ED, VOC, T = 128, 64, 2048, 512, 512, 256, 10000, 20
NCORE = 8
BC = B // NCORE            # 16 batch rows per core
BL = BC * L                # 1024
T1 = T - 1                 # 19 steps
NX = T1 * BC               # 304 rows of X per core
XD = ED + F + H            # 2816
KX = XD // 128             # 22 k-tiles for logits
START_TOK = 1

NCH = 2                    # independent pipelined chains per core
BCH = BC // NCH            # 8 rows per chain
BLC = BCH * L              # 512
KBLC = BLC // 128          # 4

KF = F // 128   # 16
KH = H // 128   # 4
KE = ED // 128  # 2
KBL = BL // 128 # 8

BF = mybir.dt.bfloat16
F32 = mybir.dt.float32
AF = mybir.ActivationFunctionType
ALU = mybir.AluOpType

_cache = {}


def _build_program(upto="all"):
    _ord = ["pre", "steps", "ctx", "all"]
    lvl = _ord.index(upto)
    nc = bacc.Bacc(None, target_bir_lowering=False, debug=False, num_devices=NCORE)

    d = {}
    def din(name, shape, dt):
        d[name] = nc.dram_tensor(name, shape, dt, kind="ExternalInput").ap()
    din("imgT", [F, BL], BF)
    din("imgN", [BL, F], BF)
    din("widx", [NX, 1], mybir.dt.int32)
    din("emb", [VOC, ED], BF)
    din("lstmk", [ED + F, 4 * H], BF)
    din("lstmrk", [H, 4 * H], BF)
    din("lstmbB", [128, 4 * H], F32)
    din("w1k", [F, U], BF)
    din("w1bT", [128, KH], F32)
    din("w2k", [H, U], BF)
    din("w2bT", [128, KH], F32)
    din("vkT", [128, KH], BF)
    din("fbkT", [128, KH], BF)
    din("fbb", [BCH, 1], F32)
    din("logitk", [XD, VOC], BF)
    din("logitbB", [128, VOC], BF)
    din("ih0k", [F, H], BF)
    din("ih0bT", [128, KH], F32)
    din("ih1k", [H, H], BF)
    din("ih1bT", [128, KH], F32)
    din("ic0k", [F, H], BF)
    din("ic0bT", [128, KH], F32)
    din("ic1k", [H, H], BF)
    din("ic1bT", [128, KH], F32)
    out_d = nc.dram_tensor("out", [NX, VOC], F32, kind="ExternalOutput").ap()

    with tile.TileContext(nc) as tc, ExitStack() as ctx:
        const = ctx.enter_context(tc.tile_pool(name="const", bufs=1))
        state = ctx.enter_context(tc.tile_pool(name="state", bufs=2))
        p2w = ctx.enter_context(tc.tile_pool(name="p2w", bufs=2))

        ident = const.tile([128, 128], BF, tag="ident")
        make_identity(nc, ident)

        w2k_t = const.tile([128, KH, U], BF, tag="w2k")
        nc.sync.dma_start(w2k_t[:], d["w2k"].rearrange("(k p) m -> p k m", p=128))
        rk_t = const.tile([128, KH, 4 * H], BF, tag="rk")
        nc.sync.dma_start(rk_t[:], d["lstmrk"].rearrange("(k p) m -> p k m", p=128))
        vk_t = const.tile([128, KH, 1], BF, tag="vk")
        nc.sync.dma_start(vk_t[:, :, 0], d["vkT"])
        fbk_t = const.tile([128, KH, 1], BF, tag="fbk")
        nc.sync.dma_start(fbk_t[:, :, 0], d["fbkT"])
        w1b_t = const.tile([128, KH], F32, tag="w1b")
        nc.sync.dma_start(w1b_t[:], d["w1bT"])
        w2b_t = const.tile([128, KH], F32, tag="w2b")
        nc.sync.dma_start(w2b_t[:], d["w2bT"])
        lstmb_t = const.tile([128, 4 * H], F32, tag="lstmb")
        nc.sync.dma_start(lstmb_t[:], d["lstmbB"])
        fbb_t = const.tile([BCH, 1], F32, tag="fbb")
        nc.sync.dma_start(fbb_t[:], d["fbb"])
        bias_t = {}
        for nm in ("ih0bT", "ih1bT", "ic0bT", "ic1bT"):
            bias_t[nm] = const.tile([128, KH], F32, tag=nm, name=nm)
            nc.sync.dma_start(bias_t[nm][:], d[nm])

        G_t = const.tile([128, KBL, 4 * H], BF, tag="G")
        featT = const.tile([128, KH, BL], BF, tag="featT")
        Wstk = const.tile([128, KBL, NX], BF, tag="Wstk")
        XT = const.tile([128, KX, NX], BF, tag="XT")
        ET = const.tile([128, KE, NX], BF, tag="ET")
        meanT = const.tile([128, KF, BC], BF, tag="meanT")
        zeA = const.tile([128, 3, 4 * H], BF, tag="zeA")   # E @ W_e + lstm_b, X-row major

        nc.any.memset(Wstk[:], 0.0)
        nc.any.memset(zeA[:, 2, :], 0.0)

        # =========== precompute: featT + mean ===========
        with tc.tile_pool(name="pc_sb", bufs=3) as pcs, \
             tc.tile_pool(name="pc_ps", bufs=1, space="PSUM") as pcp:
            mean_f = pcs.tile([128, KF, BC], F32, tag="meanf")
            fps = {}
            for n2 in range(2):
                for k in range(KF):
                    it = pcs.tile([128, BL], BF, tag="imgTt")
                    nc.sync.dma_start(it[:], d["imgT"][ts(k, 128), :])
                    if n2 == 0:
                        nc.vector.reduce_sum(
                            mean_f[:, k, :],
                            it[:].rearrange("p (b l) -> p b l", b=BC),
                            axis=mybir.AxisListType.X,
                        )
                    w1t = pcs.tile([128, U], BF, tag="w1t")
                    nc.sync.dma_start(w1t[:], d["w1k"][ts(k, 128), :])
                    for m in range(KH):
                        if k == 0:
                            fps[(n2, m)] = pcp.tile([128, 512], F32, tag=f"fp{m}", name=f"fp{m}")
                        nc.tensor.matmul(
                            fps[(n2, m)][:], w1t[:, ts(m, 128)], it[:, ts(n2, 512)],
                            start=(k == 0), stop=(k == KF - 1),
                        )
                for m in range(KH):
                    nc.vector.tensor_tensor(
                        featT[:, m, ts(n2, 512)], fps[(n2, m)][:],
                        w1b_t[:, m : m + 1].to_broadcast([128, 512]), ALU.add,
                    )
            nc.vector.tensor_scalar_mul(meanT[:], mean_f[:], 1.0 / L)

        # =========== precompute: G = img @ W_c ===========
        with tc.tile_pool(name="g_sb", bufs=3) as gs, \
             tc.tile_pool(name="g_ps", bufs=1, space="PSUM") as gp:
            for n in range(4):
                gps = [gp.tile([128, 512], F32, tag=f"gp{m}", name=f"gp{m}") for m in range(KBL)]
                for k in range(KF):
                    it = gs.tile([128, BL], BF, tag="imgTt")
                    nc.sync.dma_start(it[:], d["imgT"][ts(k, 128), :])
                    wct = gs.tile([128, 512], BF, tag="wct")
                    nc.sync.dma_start(wct[:], d["lstmk"][ED + k * 128 : ED + (k + 1) * 128, ts(n, 512)])
                    for m in range(KBL):
                        nc.tensor.matmul(
                            gps[m][:], it[:, ts(m, 128)], wct[:],
                            start=(k == 0), stop=(k == KF - 1),
                        )
                for m in range(KBL):
                    nc.vector.tensor_copy(G_t[:, m, ts(n, 512)], gps[m][:])

        # =========== init h / c (per chain) ===========
        hT = {}
        c_st = {}
        with tc.tile_pool(name="i_sb", bufs=3) as isb, \
             tc.tile_pool(name="i_ps", bufs=1, space="PSUM") as ips:
            ih1k_t = isb.tile([128, KH, H], BF, tag="ih1k", bufs=1)
            nc.sync.dma_start(ih1k_t[:], d["ih1k"].rearrange("(k p) m -> p k m", p=128))
            ic1k_t = isb.tile([128, KH, H], BF, tag="ic1k", bufs=1)
            nc.sync.dma_start(ic1k_t[:], d["ic1k"].rearrange("(k p) m -> p k m", p=128))
            for which in range(2):  # 0: h, 1: c
                k0, b0n, k1, b1n = (
                    ("ih0k", "ih0bT", ih1k_t, "ih1bT") if which == 0
                    else ("ic0k", "ic0bT", ic1k_t, "ic1bT")
                )
                b0, b1 = bias_t[b0n], bias_t[b1n]
                a1 = isb.tile([128, KH, BC], BF, tag="a1", name=f"a1_{which}")
                ps1 = [ips.tile([128, BC], F32, tag=f"ip{m}", name=f"ip{which}{m}") for m in range(KH)]
                for k in range(KF):
                    w0t = isb.tile([128, H], BF, tag="w0t", name=f"w0t_{which}_{k}")
                    nc.sync.dma_start(w0t[:], d[k0][ts(k, 128), :])
                    for m in range(KH):
                        nc.tensor.matmul(
                            ps1[m][:], w0t[:, ts(m, 128)], meanT[:, k, :],
                            start=(k == 0), stop=(k == KF - 1),
                        )
                for m in range(KH):
                    nc.scalar.activation(a1[:, m, :], ps1[m][:], AF.Relu, bias=b0[:, m : m + 1])
                ps2 = [ips.tile([128, BC], F32, tag=f"ip{m}", name=f"iq{which}{m}") for m in range(KH)]
                for k in range(KH):
                    for m in range(KH):
                        nc.tensor.matmul(
                            ps2[m][:], k1[:, k, ts(m, 128)], a1[:, k, :],
                            start=(k == 0), stop=(k == KH - 1),
                        )
                sT = isb.tile([128, KH, BC], BF, tag="sT", name=f"sT{which}")
                for m in range(KH):
                    nc.scalar.activation(sT[:, m, :], ps2[m][:], AF.Relu, bias=b1[:, m : m + 1])
                    nc.scalar.activation(sT[:, m, :], sT[:, m, :], AF.Tanh)
                if which == 0:
                    for ch in range(NCH):
                        hT[ch] = state.tile([128, KH, BCH], BF, tag=f"hT{ch}", name=f"hT{ch}")
                        nc.vector.tensor_copy(hT[ch][:], sT[:, :, ds(ch * BCH, BCH)])
                else:
                    for ch in range(NCH):
                        c_st[ch] = state.tile([BCH, H], F32, tag=f"c{ch}", name=f"c{ch}")
                        for m in range(KH):
                            tp = ips.tile([BCH, 128], BF, tag="ctr", name=f"ctr{ch}{m}")
                            nc.tensor.transpose(tp[:], sT[:, m, ds(ch * BCH, BCH)], ident[:])
                            nc.vector.tensor_copy(c_st[ch][:, ts(m, 128)], tp[:])

        # =========== embedding gather -> ET ===========
        with tc.tile_pool(name="e_sb", bufs=2) as esb, \
             tc.tile_pool(name="e_ps", bufs=2, space="PSUM") as eps:
            for r in range(3):
                rows = min(128, NX - r * 128)
                it = esb.tile([128, 1], mybir.dt.int32, tag="eidx", name=f"eidx{r}")
                nc.sync.dma_start(it[:rows], d["widx"][ds(r * 128, rows), :])
                eg = esb.tile([128, ED], BF, tag="eg", name=f"eg{r}")
                nc.gpsimd.indirect_dma_start(
                    out=eg[:rows], out_offset=None,
                    in_=d["emb"][:],
                    in_offset=bass.IndirectOffsetOnAxis(ap=it[:rows, :1], axis=0),
                )
                for cc in range(KE):
                    tp = eps.tile([128, 128], BF, tag="etr", name=f"etr{r}{cc}")
                    nc.tensor.transpose(tp[:, :rows], eg[:rows, ts(cc, 128)], ident[:rows, :rows])
                    nc.vector.tensor_copy(ET[:, cc, ds(r * 128, rows)], tp[:, :rows])

        # =========== ze_all = E @ W_e + lstm_b (batched over t) ===========
        with tc.tile_pool(name="z_ps", bufs=2, space="PSUM") as zps, \
             tc.tile_pool(name="z_sb", bufs=1) as zsb:
            we_t = zsb.tile([128, KE, 4 * H], BF, tag="we")
            nc.sync.dma_start(we_t[:], d["lstmk"][0:ED, :].rearrange("(k p) m -> p k m", p=128))
            for m in range(3):
                rows = min(128, NX - m * 128)
                for n in range(4):
                    pz0 = zps.tile([128, 512], F32, tag="pz0", name=f"zeA{m}_{n}")
                    for k in range(KE):
                        nc.tensor.matmul(
                            pz0[:rows], ET[:, k, ds(m * 128, rows)], we_t[:, k, ts(n, 512)],
                            start=(k == 0), stop=(k == KE - 1),
                        )
                    nc.vector.tensor_tensor(
                        zeA[:rows, m, ts(n, 512)], pz0[:rows],
                        lstmb_t[:rows, ts(n, 512)], ALU.add,
                    )

        # =========== recurrent steps (NCH pipelined chains) ===========
        if lvl >= 1:
            with tc.tile_pool(name="s_sb", bufs=1) as ssb, \
                 tc.tile_pool(name="s_ps", bufs=1, space="PSUM") as sps:
                gate_order = [0, 1, 3, 2]          # i, f, o (Sigmoid) then g (Tanh)
                gate_fn = {0: AF.Sigmoid, 1: AF.Sigmoid, 2: AF.Tanh, 3: AF.Sigmoid}

                def emit_h1(t, ch):
                    if True:
                        sfx = f"_{t}_{ch}"
                        kk0 = ch * KBLC
                        cb = t * BC + ch * BCH      # column base in NX space
                        # S1+S2: hid_projT psum, then tanh(featT + ph bcast)
                        # (W2_b is folded into featT's bias at host)
                        tnh = ssb.tile([128, KH, BLC], BF, tag=f"tnh{ch}", bufs=2, name="tnh" + sfx)
                        for m in range(KH):
                            ph = sps.tile([128, BCH], F32, tag="phsc", bufs=2, name="ph" + sfx + f"_{m}")
                            for k in range(KH):
                                nc.tensor.matmul(
                                    ph[:], w2k_t[:, k, ts(m, 128)], hT[ch][:, k, :],
                                    start=(k == 0), stop=(k == KH - 1),
                                )
                            nc.vector.tensor_tensor(
                                tnh[:, m, :].rearrange("p (b l) -> p b l", b=BCH),
                                featT[:, m, ds(ch * BLC, BLC)].rearrange("p (b l) -> p b l", b=BCH),
                                ph[:, :, None].to_broadcast([128, BCH, L]),
                                ALU.add,
                            )
                            nc.scalar.activation(tnh[:, m, :], tnh[:, m, :], AF.Tanh)

                        # S3: scores = V.T @ tanh -> [1, BLC] (stays in PSUM)
                        psc = sps.tile([1, BLC], F32, tag="phsc", bufs=2, name="psc" + sfx)
                        for k in range(KH):
                            nc.tensor.matmul(
                                psc[:], vk_t[:, k, :], tnh[:, k, :],
                                start=(k == 0), stop=(k == KH - 1),
                            )

                        # S4: beta = sigmoid(h @ fbeta + b) -> [BCH, 1]
                        pb = sps.tile([BCH, 1], F32, tag="tiny", bufs=2, name="pb" + sfx)
                        for k in range(KH):
                            nc.tensor.matmul(
                                pb[:], hT[ch][:, k, :], fbk_t[:, k, :],
                                start=(k == 0), stop=(k == KH - 1),
                            )
                        beta = ssb.tile([BCH, 1], F32, tag=f"beta{ch}", bufs=2, name="beta" + sfx)
                        nc.scalar.activation(beta[:], pb[:], AF.Sigmoid, bias=fbb_t[:, 0:1])

                        # S5: sigmoid(scores) on ACT straight from PSUM, DMA
                        # to [b, l] layout, then e^x = sig/(1-sig) and
                        # beta/sum folding on cheap [8, 64] DVE ops
                        sg = ssb.tile([1, BLC], F32, tag=f"sg{ch}", bufs=2, name="sg" + sfx)
                        nc.scalar.activation(sg[:], psc[:], AF.Sigmoid)
                        sg8 = ssb.tile([BCH, L], F32, tag=f"sg8{ch}", bufs=2, name="sg8" + sfx)
                        nc.sync.dma_start(sg8[:], sg[:].rearrange("o (b l) -> o b l", b=BCH))
                        om = ssb.tile([BCH, L], F32, tag=f"om{ch}", bufs=2, name="om" + sfx)
                        nc.vector.tensor_scalar(om[:], sg8[:], -1.0, 1.0, ALU.mult, ALU.add)
                        nc.vector.reciprocal(om[:], om[:])
                        nc.vector.tensor_tensor(sg8[:], sg8[:], om[:], ALU.mult)  # e^x
                        sm = ssb.tile([BCH, 1], F32, tag=f"sm{ch}", bufs=2, name="sm" + sfx)
                        nc.vector.reduce_sum(sm[:], sg8[:], axis=mybir.AxisListType.X)
                        nc.vector.reciprocal(sm[:], sm[:])
                        fac = ssb.tile([BCH, 1], F32, tag=f"fac{ch}", bufs=2, name="fac" + sfx)
                        nc.vector.tensor_tensor(fac[:], beta[:], sm[:], ALU.mult)
                        wb = ssb.tile([BCH, L], BF, tag=f"wb{ch}", bufs=2, name="wb" + sfx)
                        nc.vector.tensor_scalar_mul(wb[:], sg8[:], fac[:, 0:1])
                        ptw = sps.tile([L, BCH], BF, tag="tiny", bufs=2, name="ptw" + sfx)
                        nc.tensor.transpose(ptw[:], wb[:], ident[:BCH, :BCH])
                        # block-diag scatter into this chain's k-tile band
                        wf_lo = Wstk[0:64, ds(kk0, KBLC), :].rearrange("p k n -> p (k n)")
                        wf_hi = Wstk[64:128, ds(kk0, KBLC), :].rearrange("p k n -> p (k n)")
                        # col for global k-tile kk is t*BC + 2*kk (+1 for the
                        # odd-b half); within the chain's flattened band the
                        # element at k' sits at k'*(NX+2) + t*BC + 2*kk0
                        st0 = t * BC + 2 * kk0
                        wlb = ssb.tile([L, BCH], BF, tag=f"wlb{ch}", bufs=2, name="wlb" + sfx)
                        nc.vector.tensor_copy(wlb[:], ptw[:])
                        nc.sync.dma_start(wf_lo[:, st0 :: NX + 2], wlb[:, 0::2])
                        nc.sync.dma_start(wf_hi[:, st0 + 1 :: NX + 2], wlb[:, 1::2])

                def emit_h2(t, ch):
                    if True:
                        sfx = f"_{t}_{ch}"
                        kk0 = ch * KBLC
                        cb = t * BC + ch * BCH
                        # S6+S7: z = Wstk.T @ G + h.T @ rk + e.T @ We (+b) -> gates
                        gates = {}
                        zrow = t * BC + ch * BCH
                        zm, zoff = zrow // 128, zrow % 128
                        for n in gate_order:
                            pz = sps.tile([BCH, 512], F32, tag="pz", bufs=2, name="pz" + sfx + f"_{n}")
                            nmm = KBLC + KH + 1
                            i_mm = 0
                            for k in range(KBLC):
                                nc.tensor.matmul(
                                    pz[:], Wstk[:, kk0 + k, ds(cb, BCH)], G_t[:, kk0 + k, ts(n, 512)],
                                    start=(i_mm == 0), stop=(i_mm == nmm - 1))
                                i_mm += 1
                            for k in range(KH):
                                nc.tensor.matmul(
                                    pz[:], hT[ch][:, k, :], rk_t[:, k, ts(n, 512)],
                                    start=(i_mm == 0), stop=(i_mm == nmm - 1))
                                i_mm += 1
                            # += zeA rows [zrow, zrow+BCH) via identity-column selector
                            nc.tensor.matmul(
                                pz[:], ident[:, ds(zoff, BCH)], zeA[:, zm, ts(n, 512)],
                                start=(i_mm == 0), stop=(i_mm == nmm - 1))
                            g = ssb.tile([BCH, 512], BF, tag=f"gate{n}{ch}", name=f"gate{n}" + sfx)
                            nc.scalar.activation(g[:], pz[:], gate_fn[n])
                            gates[n] = g

                        sig_i, sig_f, tanh_g, sig_o = gates[0], gates[1], gates[2], gates[3]
                        fc = ssb.tile([BCH, H], F32, tag=f"fc{ch}", name="fc" + sfx)
                        nc.vector.tensor_tensor(fc[:], sig_f[:], c_st[ch][:], ALU.mult)
                        ig = ssb.tile([BCH, H], F32, tag=f"ig{ch}", name="ig" + sfx)
                        nc.vector.tensor_tensor(ig[:], sig_i[:], tanh_g[:], ALU.mult)
                        c_new = state.tile([BCH, H], F32, tag=f"c{ch}", name="c" + sfx)
                        nc.vector.tensor_add(c_new[:], fc[:], ig[:])
                        tc2 = ssb.tile([BCH, H], BF, tag=f"tc2{ch}", name="tc2" + sfx)
                        nc.scalar.activation(tc2[:], c_new[:], AF.Tanh)
                        h2 = ssb.tile([BCH, H], BF, tag=f"h2{ch}", bufs=2, name="h2" + sfx)
                        nc.vector.tensor_tensor(h2[:], sig_o[:], tc2[:], ALU.mult)

                        # S8: transpose h2 -> new hT; also into XT rows 2304:2816
                        hT_new = state.tile([128, KH, BCH], BF, tag=f"hT{ch}", name="hTn" + sfx)
                        for k in range(KH):
                            pt = sps.tile([128, BCH], BF, tag="pt", bufs=2, name="pt" + sfx + f"_{k}")
                            nc.tensor.transpose(pt[:], h2[:, ts(k, 128)], ident[:BCH, :BCH])
                            nc.vector.tensor_copy(hT_new[:, k, :], pt[:])
                        nc.vector.tensor_copy(XT[:, 18 : 18 + KH, ds(cb, BCH)], hT_new[:])
                        hT[ch] = hT_new
                        c_st[ch] = c_new

                # staggered emission: chain B's gate burst fills the PE while
                # chain A's attention/softmax latency resolves, and vice versa
                emit_h1(0, 0)
                for t in range(T1):
                    emit_h1(t, 1)
                    emit_h2(t, 0)
                    if t + 1 < T1:
                        emit_h1(t + 1, 0)
                    emit_h2(t, 1)

        for k in range(KE):
            nc.vector.tensor_copy(XT[:, k, :], ET[:, k, :])

        # =========== CTX: XT rows 256:2304 = (img.T @ Wstk) per chain ===========
        if lvl >= 2:
            with tc.tile_pool(name="c_sb", bufs=2) as csb, \
                 tc.tile_pool(name="c_ps", bufs=1, space="PSUM") as cps:
                for ch in range(NCH):
                    kk0 = ch * KBLC
                    for half in range(2):
                        pcs_ = [cps.tile([128, T1 * BCH], F32, tag=f"cp{m}", name=f"cp{ch}{half}{m}") for m in range(8)]
                        for k in range(KBLC):
                            int_ = csb.tile([128, F], BF, tag="imgNt", name=f"imgNt{ch}{half}{k}")
                            nc.sync.dma_start(int_[:], d["imgN"][ts(kk0 + k, 128), :])
                            rhs = Wstk[:, kk0 + k, :].rearrange("p (t b) -> p t b", t=T1)[
                                :, :, ds(ch * BCH, BCH)
                            ]
                            for m in range(8):
                                mg = half * 8 + m
                                nc.tensor.matmul(
                                    pcs_[m][:], int_[:, ts(mg, 128)], rhs,
                                    start=(k == 0), stop=(k == KBLC - 1),
                                )
                        for m in range(8):
                            mg = half * 8 + m
                            nc.vector.tensor_copy(
                                XT[:, 2 + mg, :].rearrange("p (t b) -> p t b", t=T1)[
                                    :, :, ds(ch * BCH, BCH)
                                ],
                                pcs_[m][:].rearrange("p (t b) -> p t b", t=T1),
                            )

        # =========== logits ===========
        if lvl >= 3:
            NV = 20
            with tc.tile_pool(name="l_ps", bufs=2, space="PSUM") as lps, \
                 tc.tile_pool(name="l_sb", bufs=3) as lsb:
                logitk_r = d["logitk"].rearrange("(a p) v -> p a v", p=128)
                for v in range(NV):
                    vs = min(512, VOC - v * 512)
                    wv = p2w.tile([128, KX, 512], BF, tag="wv", name=f"wv{v}")
                    nc.sync.dma_start(wv[:, :, :vs], logitk_r[:, :, ds(v * 512, vs)])
                    lb = lsb.tile([128, 512], BF, tag="lb", name=f"lb{v}")
                    nc.sync.dma_start(lb[:, :vs], d["logitbB"][:, ds(v * 512, vs)])
                    for m in range(3):
                        rows = min(128, NX - m * 128)
                        pl = lps.tile([128, 512], F32, tag="pl", name=f"pl{v}_{m}")
                        for k in range(KX):
                            nc.tensor.matmul(
                                pl[:rows, :vs], XT[:, k, ds(m * 128, rows)], wv[:, k, :vs],
                                start=(k == 0), stop=(k == KX - 1),
                            )
                        ol = lsb.tile([128, 512], F32, tag="ol", name=f"ol{v}_{m}")
                        nc.vector.tensor_add(ol[:rows, :vs], pl[:rows, :vs], lb[:rows, :vs])
                        nc.sync.dma_start(
                            out_d[ds(m * 128, rows), ds(v * 512, vs)], ol[:rows, :vs]
                        )

    nc.compile()
    return nc


def _bf(x):
    return np.ascontiguousarray(np.asarray(x, dtype=np.float32)).astype(ml_dtypes.bfloat16)


def _f32(x):
    return np.ascontiguousarray(np.asarray(x, dtype=np.float32))


def kernel(
    img_tensor, target, emb, W1_k, W1_b, W2_k, W2_b, V_k, V_b,
    fbeta_k, fbeta_b, lstm_k, lstm_rk, lstm_b, logit_k, logit_b,
    ih0_k, ih0_b, ih1_k, ih1_b, ic0_k, ic0_b, ic1_k, ic1_b,
):
    if "nc" not in _cache:
        _cache["nc"] = _build_program()
    nc = _cache["nc"]

    img = _f32(img_tensor)
    tgt = np.asarray(target)
    words = np.concatenate(
        [np.full((B, 1), START_TOK, dtype=np.int64), tgt[:, 1:-1].astype(np.int64)],
        axis=1,
    ).T  # (T1, B)

    def colT(v, kk):
        return np.ascontiguousarray(_f32(v).reshape(kk, 128).T)

    shared = {
        "emb": _bf(emb),
        "lstmk": _bf(lstm_k),
        "lstmrk": _bf(lstm_rk),
        "lstmbB": np.tile(_f32(lstm_b)[None, :], (128, 1)),
        "w1k": _bf(W1_k),
        "w1bT": colT(_f32(W1_b) + _f32(W2_b), U // 128),
        "w2k": _bf(W2_k),
        "w2bT": colT(W2_b, U // 128),
        "vkT": _bf(colT(np.asarray(V_k).reshape(-1), U // 128)),
        "fbkT": _bf(colT(np.asarray(fbeta_k).reshape(-1), H // 128)),
        "fbb": np.full((BCH, 1), float(np.asarray(fbeta_b).reshape(-1)[0]), np.float32),
        "logitk": _bf(logit_k),
        "logitbB": np.tile(_bf(logit_b)[None, :], (128, 1)),
        "ih0k": _bf(ih0_k), "ih0bT": colT(ih0_b, H // 128),
        "ih1k": _bf(ih1_k), "ih1bT": colT(ih1_b, H // 128),
        "ic0k": _bf(ic0_k), "ic0bT": colT(ic0_b, H // 128),
        "ic1k": _bf(ic1_k), "ic1bT": colT(ic1_b, H // 128),
    }

    in_maps = []
    for c in range(NCORE):
        imgc = img[c * BC : (c + 1) * BC].reshape(BL, F)
        wc = words[:, c * BC : (c + 1) * BC].reshape(NX, 1)
        m = dict(shared)
        m["imgN"] = _bf(imgc)
        m["imgT"] = _bf(imgc.T)
        m["widx"] = wc.astype(np.int32)
        in_maps.append(m)

    res = run_bass_kernel_spmd(nc, in_maps, list(range(NCORE)))

    out = np.empty((B, VOC, T1), np.float32)
    for c in range(NCORE):
        oc = res.results[c]["out"].reshape(T1, BC, VOC)
        out[c * BC : (c + 1) * BC] = oc.transpose(1, 2, 0)
    return out
